# revision 1
# baseline (speedup 1.0000x reference)
"""Channel-attention module (CAM) forward for Trainium2.

Computes, per batch b:
    f1 = x[b].reshape(C, H*W)                      # [512, 4096]
    S  = f1 @ f1.T                                 # [512, 512]
    G  = softmax(S_max - S, axis=-1)               # == exp(S_min - S) / rowsum
    fc = G @ f1
    y[b] = beta * fc + x[b]

Sharding: data-parallel over batch B=16 across 8 NeuronCores (2 batches/core),
no cross-core communication. Matmuls run in bf16 on the PE array with fp32
PSUM accumulation; softmax statistics and the final residual combine are fp32.

Per-core dataflow (per batch), tuned against the Tile cost-model timeline:
  - x loads as 8 big [128, 2048] fp32 SWDGE tiles (resident: they feed the
    bf16 casts, the fc rhs casts, and the final fp32 residual add).
  - f1^T bf16 via ACT fp32->bf16 casts + 8 big DMA xbar transposes into two
    half-K buffers. All plain DMAs ride SWDGE (gpsimd) and only these 8
    transposes use HWDGE: the scheduler hard-serializes xbar-mode transitions
    between DmaTranspose and any DMACopy, so the DMA stream is arranged as
    [loads b][xposes b][loads b+1][xposes b+1][stores b][stores b+1] with
    only two mode transitions per batch.
  - S runs kh-outer / m-mid / k-inner: kh=1 transposes overlap kh=0 matmuls,
    and each row-block's softmax fires as soon as its kh=1 block finishes.
  - Softmax: DVE row-min, ACT exp (bias=rowmin, scale=-1) with fp32 row-sum
    accum_out. beta/Z is computed as beta*exp(-ln Z) entirely on ACT right
    after the exp (same-engine program order => no sequencer stalls), and is
    applied in the fused epilogue, so G stays unnormalized.
  - G^T on the PE (16 transpose-mode matmuls into freed S psum banks + 4 ACT
    psum->sbuf copies) to keep it off the DMA stream.
  - fc runs j-outer (one DVE bf16 rhs cast per (kt, j), reused by 4
    m-blocks), accumulating in 4 PSUM banks; the epilogue is one fused DVE
    scalar_tensor_tensor: y = (beta/Z)[c]*fc_raw + x, stored via SWDGE.
"""

import numpy as np

B, C, HW = 16, 512, 4096
NCORES = 8
BL = B // NCORES  # batches per core
P = 128
CT = C // P       # 4 c-tiles of 128 channels
NCH = 8           # n-chunks of 512
F = 512           # n-chunk size (psum free dim)
HALF = HW // 2    # 2048, the load/cast/transpose granularity
KH = 16           # 128-wide k-tiles per half

_CACHE = {}


def _build():
    import concourse.bass as bass  # noqa: F401
    import concourse.mybir as mybir
    import concourse.tile as tile
    from concourse import bacc
    from concourse.masks import make_identity

    f32 = mybir.dt.float32
    bf16 = mybir.dt.bfloat16
    AF = mybir.ActivationFunctionType
    OP = mybir.AluOpType
    AX = mybir.AxisListType

    # Bacc (not plain Bass): its compile() runs generate_event_semaphores,
    # which splits excess sync waits onto EventSemaphore instructions —
    # required because TRN2 instructions (notably DMA_DIRECT2D_XPOSE) carry
    # at most one wait.
    nc = bacc.Bacc("TRN2", target_bir_lowering=False, debug=False)
    x_d = nc.dram_tensor("x", [BL, C, HW], f32, kind="ExternalInput")
    beta_d = nc.dram_tensor("beta", [1], f32, kind="ExternalInput")
    y_d = nc.dram_tensor("y", [BL, C, HW], f32, kind="ExternalOutput")

    with tile.TileContext(nc) as tc:
        with (
            tc.tile_pool(name="singles", bufs=1) as singles,
            tc.tile_pool(name="xf", bufs=16) as xf,          # [128,2048] f32
            tc.tile_pool(name="stage", bufs=3) as stage_p,   # [128,2048] bf16
            tc.tile_pool(name="f1t", bufs=2) as f1t_p,       # [128,16,512] bf16
            tc.tile_pool(name="gst", bufs=2) as gst_p,       # [128,4,512] bf16
            tc.tile_pool(name="soft", bufs=4) as soft_p,     # [128,1] f32
            tc.tile_pool(name="ebf", bufs=4) as ebf_p,       # [128,512] bf16
            # 4 bufs: all four e[m] are live at the S->fc boundary; with
            # fewer, exp[m+2] stalls on the PE transposes releasing a slot
            tc.tile_pool(name="rhs", bufs=4) as rhs_p,       # [128,512] bf16
            tc.tile_pool(name="outs", bufs=3) as out_p,      # [128,1024] f32
            tc.tile_pool(name="ps_s", bufs=4, space="PSUM") as ps_s,
            tc.tile_pool(name="ps_fc", bufs=4, space="PSUM") as ps_fc,
        ):
            beta_sb = singles.tile([P, 1], f32)
            nc.gpsimd.dma_start(out=beta_sb[:], in_=beta_d[:].to_broadcast([P, 1]))
            ident = singles.tile([P, P], bf16)
            make_identity(nc, ident[:])

            from bass_rust import add_dep_helper

            # The scheduler hard-serializes every xbar-mode transition in the
            # scheduled DMA order (DmaTranspose <-> any DMACopy, both SWDGE
            # and HWDGE). Left alone it interleaves loads and transposes,
            # turning the DMA system into a ping-pong serial chain. We pin a
            # coherent segment order instead:
            #   [loads h][xposes h] per half, then next batch, then stores.
            # One cross-engine dep per boundary suffices: each engine's
            # sequencer is in-order, loads/stores issue from Pool and
            # transposes from ACT.
            last_xpose = [None]  # most recent transpose instruction

            # x loads + bf16 cast (ACT) + big DMA xbar transpose (HWDGE),
            # emitted per half so the S kh=0 matmuls can start after ~half
            # the DMA prep
            def emit_loads_prep(b):
                xts = {}
                f1t = []
                for h in range(2):
                    fh = f1t_p.tile([P, KH, F], bf16, tag="f1t", name=f"f1t_{b}_{h}")
                    last_load = None
                    for ct in range(CT):
                        t = xf.tile([P, HALF], f32, tag="xf", name=f"x_{b}_{ct}_{h}")
                        ld = nc.gpsimd.dma_start(
                            out=t[:],
                            in_=x_d[b, ct * P : (ct + 1) * P, h * HALF : (h + 1) * HALF],
                        )
                        if ct == 0 and last_xpose[0] is not None:
                            add_dep_helper(
                                ld.ins,
                                last_xpose[0],
                                reason="xbar segment: loads after prior xposes",
                            )
                        last_load = ld.ins
                        xts[(ct, h)] = t
                    for ct in range(CT):
                        st = stage_p.tile(
                            [P, HALF], bf16, tag="stage", name=f"st_{b}_{ct}_{h}"
                        )
                        nc.scalar.copy(out=st[:], in_=xts[(ct, h)][:])
                        # MUST issue from ACT: SP-issued dma_start_transpose
                        # reproducibly faults the device (the known TC5 hang
                        # that moved hwdge transposes off SP upstream)
                        xp = nc.scalar.dma_start_transpose(
                            fh[:, :, ct * P : (ct + 1) * P], st[:]
                        )
                        if ct == 0:
                            add_dep_helper(
                                xp.ins,
                                last_load,
                                reason="xbar segment: xposes after this half's loads",
                            )
                        last_xpose[0] = xp.ins
                    f1t.append(fh)
                return xts, f1t

            xt_all = {}
            f1t_all = {}
            xt_all[0], f1t_all[0] = emit_loads_prep(0)
            first_store = {b: None for b in range(BL)}

            for b in range(BL):
                xt = xt_all[b]
                f1t = f1t_all[b]

                # ---- S = f1 @ f1^T : kh-outer / m-mid / k-inner ----
                s_ps = [
                    ps_s.tile([P, F], f32, tag="s", name=f"s_ps_{b}_{m}")
                    for m in range(CT)
                ]
                es, zs, brs = [], [], []
                for h in range(2):
                    for m in range(CT):
                        for kl in range(KH):
                            nc.tensor.matmul(
                                s_ps[m][:],
                                lhsT=f1t[h][:, kl, m * P : (m + 1) * P],
                                rhs=f1t[h][:, kl, :],
                                start=(h == 0 and kl == 0),
                                stop=(h == 1 and kl == KH - 1),
                            )
                        if h == 1:
                            # row stats of S: min (softmax shift) + rowsum of
                            # exp. G stays UNNORMALIZED (exp(S_min - S)); the
                            # beta/Z row scale is folded into the epilogue.
                            mn = soft_p.tile([P, 1], f32, tag="mn", name=f"mn{b}{m}")
                            nc.vector.tensor_reduce(
                                out=mn[:], in_=s_ps[m][:], axis=AX.X, op=OP.min
                            )
                            e = ebf_p.tile([P, F], bf16, tag="e", name=f"e{b}{m}")
                            z = soft_p.tile([P, 1], f32, tag="z", name=f"z{b}{m}")
                            nc.scalar.activation(
                                out=e[:],
                                in_=s_ps[m][:],
                                func=AF.Exp,
                                bias=mn[:],
                                scale=-1.0,
                                accum_out=z[:],
                            )
                            es.append(e)
                            zs.append(z)

                # beta/Z via beta*exp(-ln Z) on ACT (a DVE reciprocal would
                # block DVE.SEQ until S completes, starving fc). Emitted after
                # all exps so the in-order ACT stream reaches exp[m] without
                # detours.
                for m in range(CT):
                    lz = soft_p.tile([P, 1], f32, tag="lz", name=f"lz{b}{m}")
                    nc.scalar.activation(out=lz[:], in_=zs[m][:], func=AF.Ln)
                    br = soft_p.tile([P, 1], f32, tag="br", name=f"br{b}{m}")
                    nc.scalar.activation(
                        out=br[:], in_=lz[:], func=AF.Exp, scale=-1.0
                    )
                    nc.scalar.mul(out=br[:], in_=br[:], mul=beta_sb[:])
                    brs.append(br)

                # ---- G^T on the PE: 4 transpose-mode matmuls per row-block
                #      into a freed S psum bank, one ACT copy out per m ----
                # xp tiles reuse the S psum slots (freed once exp[m] has read
                # S[m]), keeping 4 banks available for the fc accumulators
                gst = gst_p.tile([P, CT, C], bf16, tag="gst", name=f"gst_{b}")
                for m in range(CT):
                    xp = ps_s.tile([P, CT, P], bf16, tag="s", name=f"xp_{b}_{m}")
                    for dt in range(CT):
                        nc.tensor.transpose(
                            xp[:, dt, :], es[m][:, dt * P : (dt + 1) * P], ident[:]
                        )
                    nc.scalar.copy(out=gst[:, :, m * P : (m + 1) * P], in_=xp[:])

                # next batch's loads + casts + transposes are emitted before
                # this batch's fc so its DMA transposes precede this batch's
                # stores in the scheduled DMA order (fewer xbar-mode stalls),
                # and so PE can roll into S(b+1) right after fc(b)
                if b + 1 < BL:
                    xt_all[b + 1], f1t_all[b + 1] = emit_loads_prep(b + 1)

                # ---- fc = G_raw @ f1 ; y = (beta/Z) * fc_raw + x ----
                for j in range(NCH):
                    h, jj = j // 4, j // 2
                    jo = (j % 4) * F  # offset within the half-tile
                    for kt in range(CT):
                        rt = rhs_p.tile([P, F], bf16, tag="rhs", name=f"rt_{b}_{j}_{kt}")
                        # split across DVE/ACT so neither paces the fc phase
                        eng = nc.vector.tensor_copy if kt < 2 else nc.scalar.copy
                        eng(out=rt[:], in_=xt[(kt, h)][:, jo : jo + F])
                        xt[("rt", kt)] = rt
                    oo = (j % 2) * F  # offset within the out-pair tile
                    f_all = [
                        ps_fc.tile([P, F], f32, tag="fc", name=f"f_ps_{b}_{j}_{m}")
                        for m in range(CT)
                    ]
                    for kt in range(CT):
                        for m in range(CT):
                            nc.tensor.matmul(
                                f_all[m][:],
                                lhsT=gst[:, kt, m * P : (m + 1) * P],
                                rhs=xt[("rt", kt)][:],
                                start=(kt == 0),
                                stop=(kt == CT - 1),
                            )
                    for m in range(CT):
                        f_ps = f_all[m]
                        if j % 2 == 0:
                            ot = out_p.tile(
                                [P, 2 * F], f32, tag="out", name=f"ot_{b}_{jj}_{m}"
                            )
                            xt[("out", jj, m)] = ot
                        else:
                            ot = xt[("out", jj, m)]
                        # y = (beta/Z)[c] * fc_raw + x, one fused DVE op
                        nc.vector.scalar_tensor_tensor(
                            out=ot[:, oo : oo + F],
                            in0=f_ps[:],
                            scalar=brs[m][:],
                            in1=xt[(m, h)][:, jo : jo + F],
                            op0=OP.mult,
                            op1=OP.add,
                        )
                        del f_ps
                        if j % 2 == 1:
                            sti = nc.gpsimd.dma_start(
                                out=y_d[
                                    b,
                                    m * P : (m + 1) * P,
                                    jj * 2 * F : (jj + 1) * 2 * F,
                                ],
                                in_=ot[:],
                            )
                            if first_store[b] is None:
                                first_store[b] = sti.ins
                                # stores come after the final xpose segment
                                add_dep_helper(
                                    sti.ins,
                                    last_xpose[0],
                                    reason="xbar segment: stores after all xposes",
                                )
    nc.finalize()
    return nc


def _get_nc():
    if "nc" not in _CACHE:
        _CACHE["nc"] = _build()
    return _CACHE["nc"]


def kernel(x: np.ndarray, beta: np.ndarray, **kw) -> np.ndarray:
    from concourse.bass_utils import run_bass_kernel_spmd

    x = np.ascontiguousarray(np.asarray(x, dtype=np.float32))
    beta = np.ascontiguousarray(np.asarray(beta, dtype=np.float32))
    assert x.shape == (B, C, 64, 64), x.shape

    xr = x.reshape(B, C, HW)
    in_maps = [
        {"x": np.ascontiguousarray(xr[i * BL : (i + 1) * BL]), "beta": beta}
        for i in range(NCORES)
    ]
    nc = _get_nc()
    res = run_bass_kernel_spmd(nc, in_maps, core_ids=list(range(NCORES)))
    out = np.concatenate([r["y"] for r in res.results], axis=0)
    return out.reshape(B, C, 64, 64).astype(np.float32)

